# revision 15
# baseline (speedup 1.0000x reference)
"""Chamfer distance kernel for Trainium2 (Bass/Tile), 8 NeuronCores.

Problem: B=16 batches of point-cloud pairs (N=M=4096 points, 3-D).
  d[b,n,m] = |x1[b,n]|^2 + |x2[b,m]|^2 - 2*x1[b,n].x2[b,m]
  dist1/idx1 = min/argmin over m, dist2/idx2 = min/argmin over n.

Sharding: data-parallel over batch; each of the 8 cores handles 2 batches.

Device algorithm (per core, per batch):
  Features A = -[x; y; z; 1; |p|^2] for cloud-1, B = [-2x; -2y; -2z; |p|^2; 1]
  for cloud-2, so a single K=5 fp32 matmul produces NEGATED squared
  distances: (A^T B)[n,m] = -d[n,m] and (B^T A)[m,n] = -d[m,n].
  PE computes -d in [128, 512] PSUM tiles; ScalarE copies tiles into a
  [128, 4096] SBUF row buffer; VectorE max8 + max_index give the row max
  (= -min d) and the FIRST index attaining it (ties -> lowest index, which
  matches jnp.argmin).
"""

import sys

import numpy as np

for _p in ("/opt/trn_rl_repo", "/root/.axon_site/_ro/trn_rl_repo"):
    if _p not in sys.path:
        sys.path.append(_p)

B, N, M, D = 16, 4096, 4096, 3
NCORES = 8
BPC = B // NCORES          # batches per core
PT = 128                   # partition tile (output rows per matmul)
FC = 512                   # free-dim chunk (PSUM bank width in fp32)
NT = N // PT               # 32 row tiles
NC_ = M // FC              # 8 chunks per row
KF = 45                    # 5 features x 9 bf16-split cross terms (exact fp32)

_CACHE = {}


def _build_program():
    import concourse.mybir as mybir
    from concourse import bacc, tile

    fp32 = mybir.dt.float32
    bf16 = mybir.dt.bfloat16
    u32 = mybir.dt.uint32

    nc = bacc.Bacc(None, target_bir_lowering=False)

    a_dram = nc.dram_tensor("feat_a", [KF, BPC * N], bf16, kind="ExternalInput")
    b_dram = nc.dram_tensor("feat_b", [KF, BPC * M], bf16, kind="ExternalInput")
    dist1_dram = nc.dram_tensor("dist1", [BPC, N], fp32, kind="ExternalOutput")
    idx1_dram = nc.dram_tensor("idx1", [BPC, N], u32, kind="ExternalOutput")
    dist2_dram = nc.dram_tensor("dist2", [BPC, M], fp32, kind="ExternalOutput")
    idx2_dram = nc.dram_tensor("idx2", [BPC, M], u32, kind="ExternalOutput")

    with tile.TileContext(nc) as tc:
        with (
            tc.tile_pool(name="feat", bufs=1) as featp,
            tc.tile_pool(name="rows", bufs=4) as rowp,
            tc.tile_pool(name="res", bufs=2) as resp,
            tc.tile_pool(name="mm", bufs=2, space="PSUM") as mmp,
        ):
            a_sb = featp.tile([KF, BPC * N], bf16, tag="a")
            b_sb = featp.tile([KF, BPC * M], bf16, tag="b")
            nc.sync.dma_start(a_sb[:], a_dram[:])
            nc.sync.dma_start(b_sb[:], b_dram[:])

            for bi in range(BPC):
                for orient in range(2):
                    if orient == 0:
                        lhs_all = a_sb[:, bi * N:(bi + 1) * N]
                        rhs_all = b_sb[:, bi * M:(bi + 1) * M]
                        vdram, idram = dist1_dram, idx1_dram
                    else:
                        lhs_all = b_sb[:, bi * M:(bi + 1) * M]
                        rhs_all = a_sb[:, bi * N:(bi + 1) * N]
                        vdram, idram = dist2_dram, idx2_dram

                    vraw = resp.tile([PT, NT * 8], fp32, tag="vraw")
                    iraw = resp.tile([PT, NT * 8], u32, tag="iraw")

                    for t in range(NT):
                        rowbuf = rowp.tile([PT, NC_ * FC], fp32, tag="rowbuf")
                        lhsT = lhs_all[:, t * PT:(t + 1) * PT]
                        # 2 PSUM tiles of 4 banks each; 4 matmuls fill one tile
                        for h in range(2):
                            mm = mmp.tile([PT, 4 * FC], fp32, tag="mm")
                            for q in range(4):
                                c = h * 4 + q
                                nc.tensor.matmul(
                                    mm[:, q * FC:(q + 1) * FC],
                                    lhsT,
                                    rhs_all[:, c * FC:(c + 1) * FC],
                                    start=True,
                                    stop=True,
                                )
                            nc.scalar.copy(
                                rowbuf[:, h * 4 * FC:(h + 1) * 4 * FC], mm[:]
                            )
                        nc.vector.max(vraw[:, t * 8:(t + 1) * 8], rowbuf[:])
                        nc.vector.max_index(
                            iraw[:, t * 8:(t + 1) * 8],
                            vraw[:, t * 8:(t + 1) * 8],
                            rowbuf[:],
                        )

                    vfin = resp.tile([PT, NT], fp32, tag="vfin")
                    vsel = vraw.rearrange("p (t e) -> p t e", e=8)[:, :, 0]
                    nc.vector.tensor_scalar_mul(vfin[:], vsel, -1.0)
                    isel = iraw.rearrange("p (t e) -> p t e", e=8)[:, :, 0]
                    nc.sync.dma_start(
                        vdram[bi].rearrange("(t p) -> p t", p=PT), vfin[:]
                    )
                    nc.sync.dma_start(
                        idram[bi].rearrange("(t p) -> p t", p=PT), isel
                    )

    nc.compile()
    return nc


def _split3(x):
    """Exact 3-way bf16 decomposition of fp32: x == h + m + l."""
    import ml_dtypes

    bf = ml_dtypes.bfloat16
    h = x.astype(bf)
    r1 = (x - h.astype(np.float32)).astype(np.float32)
    m = r1.astype(bf)
    r2 = (r1 - m.astype(np.float32)).astype(np.float32)
    l = r2.astype(bf)
    return h, m, l


def _features(x1, x2):
    """Per-batch feature matrices (KF=45 bf16 rows) such that the K-contracted
    matmul A^T B reproduces the fp32 product sum exactly: for each of the 5
    base features f, rows (f,i,j) hold splitA_i[f] / splitB_j[f] so that
    sum_ij Ai*Bj == A[f]*B[f] with bf16-exact cross products."""
    x1 = np.ascontiguousarray(x1, dtype=np.float32)
    x2 = np.ascontiguousarray(x2, dtype=np.float32)
    sq1 = (x1 * x1).sum(-1, dtype=np.float32)       # (B, N)
    sq2 = (x2 * x2).sum(-1, dtype=np.float32)       # (B, M)
    ones1 = np.ones_like(sq1)
    ones2 = np.ones_like(sq2)
    # A[b] rows: [-x, -y, -z, -1, -sq1]  (B, 5, N)
    A = -np.stack([x1[..., 0], x1[..., 1], x1[..., 2], ones1, sq1], axis=1)
    # B[b] rows: [-2x', -2y', -2z', sq2, 1]  (B, 5, M)
    Bf = np.stack(
        [-2.0 * x2[..., 0], -2.0 * x2[..., 1], -2.0 * x2[..., 2], sq2, ones2],
        axis=1,
    ).astype(np.float32)
    A = A.astype(np.float32)

    Ah, Am, Al = _split3(A)           # each (B, 5, N) bf16
    Bh, Bm, Bl = _split3(Bf)
    Asp = np.stack([Ah, Am, Al], axis=2)   # (B, 5, 3, N)
    Bsp = np.stack([Bh, Bm, Bl], axis=2)   # (B, 5, 3, M)
    # Accumulate smallest-magnitude cross terms first (PSUM adds in K order)
    order = [(2, 2), (1, 2), (2, 1), (1, 1), (0, 2), (2, 0), (0, 1), (1, 0),
             (0, 0)]
    A45 = np.concatenate([Asp[:, :, i, :] for (i, j) in order], axis=1)
    B45 = np.concatenate([Bsp[:, :, j, :] for (i, j) in order], axis=1)
    return A45, B45


def _run(input1, input2, trace=False):
    from concourse.bass_utils import run_bass_kernel_spmd

    if "nc" not in _CACHE:
        _CACHE["nc"] = _build_program()
    nc = _CACHE["nc"]

    A, Bf = _features(np.asarray(input1), np.asarray(input2))

    in_maps = []
    for c in range(NCORES):
        sl = slice(c * BPC, (c + 1) * BPC)
        # (BPC, KF, N) -> (KF, BPC*N) with [k, b*N + n] layout
        a_np = np.ascontiguousarray(A[sl].transpose(1, 0, 2).reshape(KF, BPC * N))
        b_np = np.ascontiguousarray(Bf[sl].transpose(1, 0, 2).reshape(KF, BPC * M))
        in_maps.append({"feat_a": a_np, "feat_b": b_np})

    res = run_bass_kernel_spmd(nc, in_maps, list(range(NCORES)), trace=trace)

    dist1 = np.empty((B, N), np.float32)
    dist2 = np.empty((B, M), np.float32)
    idx1 = np.empty((B, N), np.int32)
    idx2 = np.empty((B, M), np.int32)
    for c in range(NCORES):
        sl = slice(c * BPC, (c + 1) * BPC)
        r = res.results[c]
        dist1[sl] = r["dist1"]
        dist2[sl] = r["dist2"]
        idx1[sl] = r["idx1"].astype(np.int32)
        idx2[sl] = r["idx2"].astype(np.int32)
    return (dist1, dist2, idx1, idx2), res


def kernel(input1, input2):
    outs, _ = _run(input1, input2, trace=False)
    return outs


def kernel_profiled(input1, input2):
    outs, res = _run(input1, input2, trace=True)
    return outs, res


# revision 18
# speedup vs baseline: 1.0080x; 1.0080x over previous
"""Chamfer distance kernel for Trainium2 (Bass/Tile), 8 NeuronCores.

Problem: B=16 batches of point-cloud pairs (N=M=4096 points, 3-D).
  d[b,n,m] = |x1[b,n]|^2 + |x2[b,m]|^2 - 2*x1[b,n].x2[b,m]
  dist1/idx1 = min/argmin over m, dist2/idx2 = min/argmin over n.

Sharding: data-parallel over batch; each of the 8 cores handles 2 batches.

Device algorithm (per core, per batch):
  Features A = -[x; y; z; 1; |p|^2] for cloud-1, B = [-2x; -2y; -2z; |p|^2; 1]
  for cloud-2, so a single K-contracted matmul produces NEGATED squared
  distances: (A^T B)[n,m] = -d[n,m] and (B^T A)[m,n] = -d[m,n]. Each fp32
  feature is decomposed exactly into 3 bf16 components (h+m+l) and all 9
  cross terms are stacked along K (KF=45), so the bf16 matmul reproduces
  fp32-accurate products at 4x the fp32 PE rate (bf16 cross products are
  exact in fp32; only the PSUM accumulation rounds, smallest terms first).
  PE writes -d into [128, 2048] PSUM tiles; ScalarE copies them into a
  [128, 4096] SBUF row buffer; VectorE max8 + max_index give the row max
  (= -min d) and the FIRST index attaining it (ties -> lowest index, which
  matches jnp.argmin).
"""

import sys

import numpy as np

for _p in ("/opt/trn_rl_repo", "/root/.axon_site/_ro/trn_rl_repo"):
    if _p not in sys.path:
        sys.path.append(_p)

B, N, M, D = 16, 4096, 4096, 3
NCORES = 8
BPC = B // NCORES          # batches per core
PT = 128                   # partition tile (output rows per matmul)
FC = 512                   # free-dim chunk (PSUM bank width in fp32)
NT = N // PT               # 32 row tiles
NC_ = M // FC              # 8 chunks per row
KF = 45                    # 5 features x 9 bf16-split cross terms (exact fp32)

_CACHE = {}


def _build_program():
    import concourse.mybir as mybir
    from concourse import bacc, tile

    fp32 = mybir.dt.float32
    bf16 = mybir.dt.bfloat16
    u32 = mybir.dt.uint32

    nc = bacc.Bacc(None, target_bir_lowering=False)

    a_dram = nc.dram_tensor("feat_a", [KF, BPC * N], bf16, kind="ExternalInput")
    b_dram = nc.dram_tensor("feat_b", [KF, BPC * M], bf16, kind="ExternalInput")
    dist1_dram = nc.dram_tensor("dist1", [BPC, N], fp32, kind="ExternalOutput")
    idx1_dram = nc.dram_tensor("idx1", [BPC, N], u32, kind="ExternalOutput")
    dist2_dram = nc.dram_tensor("dist2", [BPC, M], fp32, kind="ExternalOutput")
    idx2_dram = nc.dram_tensor("idx2", [BPC, M], u32, kind="ExternalOutput")

    with tile.TileContext(nc) as tc:
        with (
            tc.tile_pool(name="feat", bufs=1) as featp,
            tc.tile_pool(name="rows", bufs=6) as rowp,
            tc.tile_pool(name="res", bufs=3) as resp,
            tc.tile_pool(name="mm", bufs=2, space="PSUM") as mmp,
        ):
            a_sb = featp.tile([KF, BPC * N], bf16, tag="a")
            b_sb = featp.tile([KF, BPC * M], bf16, tag="b")
            nc.sync.dma_start(a_sb[:], a_dram[:])
            nc.sync.dma_start(b_sb[:], b_dram[:])

            for bi in range(BPC):
                for orient in range(2):
                    if orient == 0:
                        lhs_all = a_sb[:, bi * N:(bi + 1) * N]
                        rhs_all = b_sb[:, bi * M:(bi + 1) * M]
                        vdram, idram = dist1_dram, idx1_dram
                    else:
                        lhs_all = b_sb[:, bi * M:(bi + 1) * M]
                        rhs_all = a_sb[:, bi * N:(bi + 1) * N]
                        vdram, idram = dist2_dram, idx2_dram

                    vraw = resp.tile([PT, NT * 8], fp32, tag="vraw")
                    iraw = resp.tile([PT, NT * 8], u32, tag="iraw")

                    for t in range(NT):
                        rowbuf = rowp.tile([PT, NC_ * FC], fp32, tag="rowbuf")
                        lhsT = lhs_all[:, t * PT:(t + 1) * PT]
                        # 2 PSUM tiles of 4 banks each; 4 matmuls fill one tile
                        for h in range(2):
                            mm = mmp.tile([PT, 4 * FC], fp32, tag="mm")
                            for q in range(4):
                                c = h * 4 + q
                                nc.tensor.matmul(
                                    mm[:, q * FC:(q + 1) * FC],
                                    lhsT,
                                    rhs_all[:, c * FC:(c + 1) * FC],
                                    start=True,
                                    stop=True,
                                )
                            nc.scalar.copy(
                                rowbuf[:, h * 4 * FC:(h + 1) * 4 * FC], mm[:]
                            )
                        nc.vector.max(vraw[:, t * 8:(t + 1) * 8], rowbuf[:])
                        nc.vector.max_index(
                            iraw[:, t * 8:(t + 1) * 8],
                            vraw[:, t * 8:(t + 1) * 8],
                            rowbuf[:],
                        )

                    vfin = resp.tile([PT, NT], fp32, tag="vfin")
                    vsel = vraw.rearrange("p (t e) -> p t e", e=8)[:, :, 0]
                    nc.scalar.mul(vfin[:], vsel, -1.0)
                    isel = iraw.rearrange("p (t e) -> p t e", e=8)[:, :, 0]
                    nc.sync.dma_start(
                        vdram[bi].rearrange("(t p) -> p t", p=PT), vfin[:]
                    )
                    nc.sync.dma_start(
                        idram[bi].rearrange("(t p) -> p t", p=PT), isel
                    )

    nc.compile()
    return nc


def _split3(x):
    """Exact 3-way bf16 decomposition of fp32: x == h + m + l."""
    import ml_dtypes

    bf = ml_dtypes.bfloat16
    h = x.astype(bf)
    r1 = (x - h.astype(np.float32)).astype(np.float32)
    m = r1.astype(bf)
    r2 = (r1 - m.astype(np.float32)).astype(np.float32)
    l = r2.astype(bf)
    return h, m, l


def _features(x1, x2):
    """Per-batch feature matrices (KF=45 bf16 rows) such that the K-contracted
    matmul A^T B reproduces the fp32 product sum exactly: for each of the 5
    base features f, rows (f,i,j) hold splitA_i[f] / splitB_j[f] so that
    sum_ij Ai*Bj == A[f]*B[f] with bf16-exact cross products."""
    x1 = np.ascontiguousarray(x1, dtype=np.float32)
    x2 = np.ascontiguousarray(x2, dtype=np.float32)
    sq1 = (x1 * x1).sum(-1, dtype=np.float32)       # (B, N)
    sq2 = (x2 * x2).sum(-1, dtype=np.float32)       # (B, M)
    ones1 = np.ones_like(sq1)
    ones2 = np.ones_like(sq2)
    # A[b] rows: [-x, -y, -z, -1, -sq1]  (B, 5, N)
    A = -np.stack([x1[..., 0], x1[..., 1], x1[..., 2], ones1, sq1], axis=1)
    # B[b] rows: [-2x', -2y', -2z', sq2, 1]  (B, 5, M)
    Bf = np.stack(
        [-2.0 * x2[..., 0], -2.0 * x2[..., 1], -2.0 * x2[..., 2], sq2, ones2],
        axis=1,
    ).astype(np.float32)
    A = A.astype(np.float32)

    Ah, Am, Al = _split3(A)           # each (B, 5, N) bf16
    Bh, Bm, Bl = _split3(Bf)
    Asp = np.stack([Ah, Am, Al], axis=2)   # (B, 5, 3, N)
    Bsp = np.stack([Bh, Bm, Bl], axis=2)   # (B, 5, 3, M)
    # Accumulate smallest-magnitude cross terms first (PSUM adds in K order)
    order = [(2, 2), (1, 2), (2, 1), (1, 1), (0, 2), (2, 0), (0, 1), (1, 0),
             (0, 0)]
    A45 = np.concatenate([Asp[:, :, i, :] for (i, j) in order], axis=1)
    B45 = np.concatenate([Bsp[:, :, j, :] for (i, j) in order], axis=1)
    return A45, B45


def _run(input1, input2, trace=False):
    from concourse.bass_utils import run_bass_kernel_spmd

    if "nc" not in _CACHE:
        _CACHE["nc"] = _build_program()
    nc = _CACHE["nc"]

    A, Bf = _features(np.asarray(input1), np.asarray(input2))

    in_maps = []
    for c in range(NCORES):
        sl = slice(c * BPC, (c + 1) * BPC)
        # (BPC, KF, N) -> (KF, BPC*N) with [k, b*N + n] layout
        a_np = np.ascontiguousarray(A[sl].transpose(1, 0, 2).reshape(KF, BPC * N))
        b_np = np.ascontiguousarray(Bf[sl].transpose(1, 0, 2).reshape(KF, BPC * M))
        in_maps.append({"feat_a": a_np, "feat_b": b_np})

    res = run_bass_kernel_spmd(nc, in_maps, list(range(NCORES)), trace=trace)

    dist1 = np.empty((B, N), np.float32)
    dist2 = np.empty((B, M), np.float32)
    idx1 = np.empty((B, N), np.int32)
    idx2 = np.empty((B, M), np.int32)
    for c in range(NCORES):
        sl = slice(c * BPC, (c + 1) * BPC)
        r = res.results[c]
        dist1[sl] = r["dist1"]
        dist2[sl] = r["dist2"]
        idx1[sl] = r["idx1"].astype(np.int32)
        idx2[sl] = r["idx2"].astype(np.int32)
    return (dist1, dist2, idx1, idx2), res


def kernel(input1, input2):
    outs, _ = _run(input1, input2, trace=False)
    return outs


def kernel_profiled(input1, input2):
    outs, res = _run(input1, input2, trace=True)
    return outs, res
